# revision 38
# baseline (speedup 1.0000x reference)
"""Trainium2 Bass kernel: Conv2d [8,8,1024,1024] x [8,8,3,3] (+bias), with
the reference's roll-by-1 on H, VALID padding -> [8,8,1022,1022].

Strategy: data-parallel over the batch dim (1 image per NeuronCore, 8 cores).
The kernel is HBM-bandwidth bound; all layout work is pushed to the host so
the device sees only large dense DMAs:

  - Everything on HBM is bf16 (tolerance is 2e-2; bf16 end-to-end is ~4e-3).
  - The host pre-stages the input into the exact SBUF tile layout
    `staged_in[128, 73*1024]`: partition p = c*16+q holds row 14b+q-1 (the
    roll is folded in) of channel c for block b at columns [1024b, 1024b+1024).
    Blocks are fetched up to 8 at a time -> one dense 16KB descriptor per
    partition per dma_start (vs 2-4KB strided descriptors straight from NCHW,
    which run at ~half rate and pay a per-dma_start completion stall).
  - Per block the conv is 3 accumulating matmuls (one per W-tap j) with the
    banded weight lhsT[(c,q),(co,dx)] = filt[co,c,q-dx,j]; K=128 = 16 rows x
    8 cin, M=112 = 14 output rows x 8 cout, N = the W dim in chunks 512+510.
  - PSUM is evicted (+bias, cast to bf16) alternately by VectorE and ScalarE
    into ot[112, g*1022]; one dense dma_start per group writes
    staged_out[112, 73*1022].  The host reassembles [8,1022,1022] f32.
"""

import os
import sys

for _p in ("/opt/trn_rl_repo",):
    if _p not in sys.path and os.path.isdir(_p):
        sys.path.insert(0, _p)

import ml_dtypes
import numpy as np

import concourse.bacc as bacc
import concourse.mybir as mybir
from concourse.bass_utils import run_bass_kernel_spmd
from concourse.tile import TileContext

F32 = mybir.dt.float32
BF16 = mybir.dt.bfloat16
NP_BF16 = ml_dtypes.bfloat16

N_CORES = 8
CIN = 8
COUT = 8
KH = 3
KW = 3
H = 1024
W = 1024
HOUT = H - 2
WOUT = W - 2
D = 14                # output rows per block
R = D + 2             # input rows per block
NB = HOUT // D        # 73 blocks, exact
M = COUT * D          # 112
CHUNKS = [(0, 512), (512, 510)]
# staircase: small first groups so the PE starts ~4us in instead of waiting
# for a full 2MB prefetch; small last group to shorten the output drain.
_SIZES = [2, 3, 8, 8, 8, 8, 8, 8, 8, 8, 4]
assert sum(_SIZES) == NB
GROUPS = []
_b = 0
for _g in _SIZES:
    GROUPS.append((_b, _g))
    _b += _g
GMAX = max(_SIZES)


def build_nc(in_bufs: int = 3, out_bufs: int = 3, psum_bufs: int = 8):
    nc = bacc.Bacc("TRN2", target_bir_lowering=False, debug=False,
                   num_devices=N_CORES)
    in_d = nc.dram_tensor("staged_in", [128, NB * W], BF16,
                          kind="ExternalInput")
    # weights + f32 bias packed as two bf16 columns (bitcast on chip)
    w_d = nc.dram_tensor("wconst", [128, KW * M + 2], BF16,
                         kind="ExternalInput")
    out_d = nc.dram_tensor("staged_out", [M, NB * WOUT], BF16,
                           kind="ExternalOutput")

    with TileContext(nc) as tc:
        with (
            tc.tile_pool(name="win", bufs=1) as wpool,
            tc.tile_pool(name="inp", bufs=in_bufs) as ipool,
            tc.tile_pool(name="outp", bufs=out_bufs) as opool,
            tc.tile_pool(name="ps", bufs=psum_bufs, space="PSUM") as ppool,
        ):
            # the first input group is the critical-path transfer: issue it
            # ahead of the (small) weight-consts DMA on the SP ring
            t_first = ipool.tile([128, GMAX * W], BF16, tag="t")
            nc.sync.dma_start(
                out=t_first[0:128, 0:GROUPS[0][1] * W],
                in_=in_d[:, 0:GROUPS[0][1] * W])

            wt = wpool.tile([128, KW * M + 2], BF16, tag="wt")
            nc.sync.dma_start(out=wt[:], in_=w_d[:])
            bt = wt[0:M, KW * M:KW * M + 2].bitcast(F32)

            ev = 0
            for (b0, g) in GROUPS:
                if b0 == 0:
                    t = t_first
                else:
                    t = ipool.tile([128, GMAX * W], BF16, tag="t")
                    nc.sync.dma_start(
                        out=t[0:128, 0:g * W],
                        in_=in_d[:, b0 * W:(b0 + g) * W])
                ot = opool.tile([M, GMAX * WOUT], BF16, tag="ot")
                for i in range(g):
                    ps0 = ppool.tile([M, 512], F32, tag="ps")
                    ps1 = ppool.tile([M, 512], F32, tag="ps")
                    pss = [ps0, ps1]
                    for j in range(KW):  # j outer: lhsT shared by both chunks
                        for ci, (c0, n) in enumerate(CHUNKS):
                            nc.tensor.matmul(
                                pss[ci][0:M, 0:n],
                                lhsT=wt[:, j * M:(j + 1) * M],
                                rhs=t[0:128, i * W + c0 + j:
                                      i * W + c0 + j + n],
                                start=(j == 0),
                                stop=(j == KW - 1),
                            )
                    for ci, (c0, n) in enumerate(CHUNKS):
                        dst = ot[0:M, i * WOUT + c0:i * WOUT + c0 + n]
                        if ev % 2 == 0:
                            nc.vector.tensor_scalar_add(dst, pss[ci][0:M, 0:n],
                                                        bt[:])
                        else:
                            nc.scalar.add(dst, pss[ci][0:M, 0:n], bt[:])
                        ev += 1
                nc.scalar.dma_start(
                    out=out_d[:, b0 * WOUT:(b0 + g) * WOUT],
                    in_=ot[0:M, 0:g * WOUT])

    nc.compile()
    return nc


def make_consts(filt: np.ndarray, bias: np.ndarray):
    wconst = np.zeros((128, KW * M), np.float32)
    for j in range(KW):
        for q in range(R):
            for dx in range(D):
                i = q - dx
                if 0 <= i < KH:
                    for c in range(CIN):
                        wconst[c * R + q,
                               j * M + np.arange(COUT) * D + dx] = \
                            filt[:, c, i, j]
    packed = np.zeros((128, KW * M + 2), NP_BF16)
    packed[:, 0:KW * M] = wconst.astype(NP_BF16)
    bits = np.repeat(bias, D).astype(np.float32).view(np.uint32)
    pu16 = packed.view(np.uint16)
    pu16[0:M, KW * M] = (bits & 0xFFFF).astype(np.uint16)
    pu16[0:M, KW * M + 1] = (bits >> 16).astype(np.uint16)
    return packed


_CACHE = {}


def _get_nc():
    if "nc" not in _CACHE:
        _CACHE["nc"] = build_nc()
    return _CACHE["nc"]


def _stage_input(core_bf16: np.ndarray) -> np.ndarray:
    """[8,1024,1024] bf16 -> staged [128, 73*1024]: partition c*16+q, block b
    holds rolled row 14b+q = orig row (14b+q-1) mod 1024."""
    dev = np.concatenate([core_bf16[:, -1:, :], core_bf16], axis=1)  # 1025 rows
    s = np.lib.stride_tricks.as_strided(
        dev,
        shape=(CIN, R, NB, W),
        strides=(dev.strides[0], dev.strides[1], D * dev.strides[1],
                 dev.strides[2]))
    return np.ascontiguousarray(s.reshape(CIN * R, NB * W))


def make_in_maps(inp, filt, bias):
    wconst = make_consts(filt, bias)
    inp_b = inp.astype(NP_BF16)
    return [
        {"staged_in": _stage_input(inp_b[n]),
         "wconst": wconst}
        for n in range(N_CORES)
    ]


def unstage_output(staged: np.ndarray) -> np.ndarray:
    """[112, 73*1022] bf16 -> [8, 1022, 1022] f32 (m=co*14+dx, col b*1022+y
    -> out[co, 14b+dx, y])."""
    s = staged.reshape(COUT, D, NB, WOUT).transpose(0, 2, 1, 3)
    return s.reshape(COUT, HOUT, WOUT).astype(np.float32)


def kernel(inp: np.ndarray, filt: np.ndarray, bias: np.ndarray) -> np.ndarray:
    inp = np.asarray(inp, np.float32)
    filt = np.asarray(filt, np.float32)
    bias = np.asarray(bias, np.float32)
    nc = _get_nc()
    in_maps = make_in_maps(inp, filt, bias)
    res = run_bass_kernel_spmd(nc, in_maps, list(range(N_CORES)))
    return np.stack([unstage_output(res.results[c]["staged_out"])
                     for c in range(N_CORES)], axis=0)
